# revision 35
# baseline (speedup 1.0000x reference)
"""CIDER loss Trainium2 kernel (8 NeuronCores, data-parallel over batch).

Math (reference):
  logits = (z @ mu.T) / T          # [B, C],  T = 0.1
  pos    = logits[b, target[b]]
  lse    = logsumexp(logits, axis=1)
  loss_comp = mean(lse - pos)
  sim    = (mu @ mu.T) / T with diag masked to -inf
  loss_dis  = mean(log(1/(C-1)) + logsumexp(sim, axis=1))
  loss = ALPHA * loss_dis + LAMDA * loss_comp

v4 design (per core, B_SH = 8192 rows = 64 tiles of 128):
  - One [128,4096] f32 PSUM tile = 4 manually rotated 1024-col regions
    (subtile dep tracking orders matmul/reduce/exp per region).
  - PE: logits10 tile = zT_tile.T @ (mu.T*10) -> region slot%4.
  - DVE: per-tile tensor_reduce(max, negate) -> nm_cols column.
  - ACT: exp(l + nm) with accum_out -> per-tile row sums s_cols.
    lse rows = ln(s) - nm with ln via the exponent-bits trick (one DVE
    op, < 0.03 nats, mean-zero) -- no second ACT table load.
  - pos: loss only needs SUM(pos) = <zT, mugT> elementwise (transposed
    layout reuses zT; no separate zn upload). mugT = (10*mu[target]).T
    pre-gathered on host (pure input indexing). stt chunks w/ accum.
  - Dispersion rides the pipeline as slot 26 (125 rows); diag mask via
    an extra accumulating matmul pair (Asel.T@Bmask) on idle PE cycles.
  - All loads on the sync HWDGE ring, dma_starts emitted interleaved
    with compute so semaphore wait targets stay small.
  - Output = [128,4] per-partition partials; host does the final sums.
"""
import sys

if "/opt/trn_rl_repo" not in sys.path:
    sys.path.insert(0, "/opt/trn_rl_repo")

from contextlib import ExitStack

import numpy as np

import concourse.bass as bass
import concourse.tile as tile
from concourse import bacc, mybir
from concourse.bass_utils import run_bass_kernel_spmd

N_CORES = 8
B, D, C = 65536, 128, 1000
B_SH = B // N_CORES            # 8192 rows per core
NT = B_SH // 128               # 64 tiles of 128 rows
CD = C // N_CORES              # dispersion rows per core (125)
SCALE = 10.0                   # 1 / T
ALPHA, LAMDA = 1.0, 2.0
F32 = mybir.dt.float32
BF16 = mybir.dt.bfloat16
AX = mybir.AxisListType
ALU = mybir.AluOpType
ACTF = mybir.ActivationFunctionType

NZCH = 8                       # pos stt chunks
ZCOLS = (NT * 128) // NZCH     # 1024 cols per chunk
DISP_SLOT = 26


def _build_program():
    nc = bacc.Bacc("TRN2", target_bir_lowering=False, debug=False,
                   num_devices=N_CORES)
    t = {}
    t["zT"] = nc.dram_tensor("zT", [D, B_SH], BF16, kind="ExternalInput").ap()
    t["mugT"] = nc.dram_tensor("mugT", [D, B_SH], BF16,
                               kind="ExternalInput").ap()
    t["muTs"] = nc.dram_tensor("muTs", [D, C], BF16, kind="ExternalInput").ap()
    t["muTd"] = nc.dram_tensor("muTd", [D, CD], BF16,
                               kind="ExternalInput").ap()
    t["asel"] = nc.dram_tensor("asel", [128, CD], BF16,
                               kind="ExternalInput").ap()
    t["bmask"] = nc.dram_tensor("bmask", [128, C], BF16,
                                kind="ExternalInput").ap()
    t["out"] = nc.dram_tensor("out", [128, 4], F32, kind="ExternalOutput").ap()

    with tile.TileContext(nc) as tc, ExitStack() as ctx:
        _build_tile_program(tc, ctx, t)
    nc.compile()
    return nc


def _build_tile_program(tc, ctx, t):
    nc = tc.nc
    singles = ctx.enter_context(tc.tile_pool(name="singles", bufs=1))
    scr_pool = ctx.enter_context(tc.tile_pool(name="scr", bufs=2))
    pp_pool = ctx.enter_context(tc.tile_pool(name="pp", bufs=NZCH))
    ps_pool = ctx.enter_context(tc.tile_pool(name="ps", bufs=1, space="PSUM"))

    # PE warm-up first, emitted BEFORE any dma_start so the Tensor queue's
    # first waits reference no DMA state: dummy matmuls on a memset scratch
    # keep the PE continuously busy from the preamble so it reaches its
    # full p-state clock before the first real matmul. They write the
    # region tile 3 will overwrite with start=True.
    warm = singles.tile([128, 512], BF16)
    nc.gpsimd.memset(warm[:], 0.0)
    P = ps_pool.tile([128, 4096], F32)
    for _ in range(5):
        nc.tensor.matmul(P[:, 3 * 1024:3 * 1024 + 512], warm[:, 0:128],
                         warm[:, 0:512], start=True, stop=True)

    # Early DMAs: just what the first few tiles need.
    muTs = singles.tile([D, C], BF16)
    nc.sync.dma_start(muTs[:, 0:512], t["muTs"][:, 0:512])
    zT = singles.tile([D, B_SH], BF16)
    nc.sync.dma_start(zT[:, 0:256], t["zT"][:, 0:256])
    nc.sync.dma_start(muTs[:, 512:1000], t["muTs"][:, 512:1000])
    nc.sync.dma_start(zT[:, 256:1536], t["zT"][:, 256:1536])
    mugT = singles.tile([D, B_SH], BF16)
    muTd = singles.tile([D, CD], BF16)
    asel = singles.tile([128, CD], BF16)
    bmask = singles.tile([128, C], BF16)

    s_cols = singles.tile([128, NT + 1], F32)
    nc.vector.memset(s_cols[:], 1.0)
    nm_cols = singles.tile([128, NT + 1], F32)
    nc.vector.memset(nm_cols[:], 0.0)
    out_sb = singles.tile([128, 4], F32)
    nc.vector.memset(out_sb[:], 0.0)
    ones = singles.tile([128, 1], BF16)
    nc.vector.memset(ones[:], 1.0)

    # Remaining DMAs, staged by the slot loop below so compute emitted in
    # between keeps its DMA-semaphore wait targets small.
    def emit_late_dma(s):
        if s == 2:
            for c in range(3):
                sl = slice(1536 + c * 2048, 1536 + (c + 1) * 2048)
                nc.sync.dma_start(zT[:, sl], t["zT"][:, sl])
        elif s == 4:
            nc.sync.dma_start(zT[:, 7680:8192], t["zT"][:, 7680:8192])
            nc.sync.dma_start(muTd[:], t["muTd"][:, :])
            nc.sync.dma_start(asel[:], t["asel"][:, :])
            nc.sync.dma_start(bmask[:], t["bmask"][:, :])
        elif s == 6:
            nc.sync.dma_start(mugT[:, 0:4096], t["mugT"][:, 0:4096])
        elif s == 8:
            nc.sync.dma_start(mugT[:, 4096:8192], t["mugT"][:, 4096:8192])

    slots = list(range(DISP_SLOT - 1)) + ["disp"] + list(range(DISP_SLOT - 1,
                                                               NT))

    def emit_mm(s):
        r = (s % 4) * 1024
        item = slots[s]
        if item == "disp":
            nc.tensor.matmul(P[0:CD, r:r + 512], muTd[:, :], muTs[:, 0:512],
                             start=True, stop=False)
            nc.tensor.matmul(P[0:CD, r:r + 512], asel[:, :], bmask[:, 0:512],
                             start=False, stop=True)
            nc.tensor.matmul(P[0:CD, r + 512:r + 1000], muTd[:, :],
                             muTs[:, 512:1000], start=True, stop=False)
            nc.tensor.matmul(P[0:CD, r + 512:r + 1000], asel[:, :],
                             bmask[:, 512:1000], start=False, stop=True)
        else:
            j = item
            lhs = zT[:, j * 128:(j + 1) * 128]
            nc.tensor.matmul(P[:, r:r + 512], lhs, muTs[:, 0:512],
                             start=True, stop=True)
            nc.tensor.matmul(P[:, r + 512:r + 1000], lhs, muTs[:, 512:1000],
                             start=True, stop=True)

    def emit_red_exp(s):
        r = (s % 4) * 1024
        item = slots[s]
        np_, col = (CD, NT) if item == "disp" else (128, item)
        nc.vector.tensor_reduce(out=nm_cols[0:np_, col:col + 1],
                                in_=P[0:np_, r:r + 1000],
                                axis=AX.X, op=ALU.max, negate=True)
        scr = scr_pool.tile([128, C], BF16, tag="scr")
        nc.scalar.activation(out=scr[0:np_, :], in_=P[0:np_, r:r + 1000],
                             func=ACTF.Exp, bias=nm_cols[0:np_, col:col + 1],
                             scale=1.0, accum_out=s_cols[0:np_, col:col + 1])

    pp_ch = []

    def emit_pos_chunk(c):
        # Products on the otherwise-idle Pool engine; persisted (bufs=NZCH)
        # and folded by a PE ones-matmul burst after the main loop, so no
        # mid-stream engine waits on Pool.
        sl = slice(c * ZCOLS, (c + 1) * ZCOLS)
        pp = pp_pool.tile([128, ZCOLS], BF16, tag=f"pp{c}")
        nc.gpsimd.tensor_tensor(pp[:], zT[:, sl], mugT[:, sl], ALU.mult)
        pp_ch.append(pp)

    emit_mm(0)
    emit_mm(1)
    for s in range(len(slots)):
        emit_late_dma(s)
        if s + 2 < len(slots):
            emit_mm(s + 2)
        emit_red_exp(s)
        if s >= 12 and s % 4 == 0 and (s - 12) // 4 < NZCH:
            emit_pos_chunk((s - 12) // 4)

    # PE burst: fold the persisted pos-product chunks into a [1,512] PSUM
    # accumulator (region-1 columns, free after exp of slot 61) via the
    # ones-trick. Runs after the last main matmul on idle PE cycles.
    for c in range(NZCH):
        nc.tensor.matmul(P[0:1, 1024:1536], ones[:, 0:1], pp_ch[c][:, 0:512],
                         start=(c == 0), stop=False)
        nc.tensor.matmul(P[0:1, 1024:1536], ones[:, 0:1],
                         pp_ch[c][:, 512:1024],
                         start=False, stop=(c == NZCH - 1))

    # --- tail: lse rows = ln(s) - nm via the bits trick; ship partials.
    ln_cols = singles.tile([128, NT + 1], F32)
    nc.vector.tensor_scalar(
        out=ln_cols[:], in0=s_cols[:].bitcast(mybir.dt.int32),
        scalar1=8.262958405e-8, scalar2=-87.98998, op0=ALU.mult, op1=ALU.add)
    contrib = singles.tile([128, NT + 1], F32)
    nc.vector.tensor_sub(contrib[:], ln_cols[:], nm_cols[:])
    nc.vector.tensor_reduce(out=out_sb[:, 0:1], in_=contrib[:, 0:NT],
                            axis=AX.X, op=ALU.add)
    nc.vector.tensor_reduce(out=out_sb[0:1, 3:4], in_=P[0:1, 1024:1536],
                            axis=AX.X, op=ALU.add)
    nc.vector.tensor_copy(out_sb[0:CD, 2:3], contrib[0:CD, NT:NT + 1])
    nc.sync.dma_start(t["out"][:, :], out_sb[:])


_NC_CACHE = {}


def _get_program():
    if "nc" not in _NC_CACHE:
        _NC_CACHE["nc"] = _build_program()
    return _NC_CACHE["nc"]


def make_in_maps(z, target, mu):
    import ml_dtypes
    bf16 = ml_dtypes.bfloat16
    z = np.ascontiguousarray(np.asarray(z, dtype=np.float32))
    mu = np.ascontiguousarray(np.asarray(mu, dtype=np.float32))
    target = np.asarray(target).astype(np.int64)
    muTs = np.ascontiguousarray((mu.T * np.float32(SCALE)).astype(bf16))
    muT_bf = np.ascontiguousarray(mu.T.astype(bf16))           # [128, 1000]
    asel = np.zeros((128, CD), dtype=np.float32)
    asel[np.arange(CD), np.arange(CD)] = 1.0
    asel = asel.astype(bf16)
    mu_rows10 = (mu * np.float32(SCALE)).astype(bf16)          # [1000, 128]
    in_maps = []
    for k in range(N_CORES):
        zs = z[k * B_SH:(k + 1) * B_SH]                        # [8192, 128]
        zT = np.ascontiguousarray(zs.T.astype(bf16))           # [128, 8192]
        ts = target[k * B_SH:(k + 1) * B_SH]
        mugT = np.ascontiguousarray(mu_rows10[ts].T)           # [128, 8192]
        bmask = np.zeros((128, C), dtype=np.float32)
        bmask[np.arange(CD), k * CD + np.arange(CD)] = np.float32(-1e30)
        in_maps.append({
            "zT": zT,
            "mugT": mugT,
            "muTs": muTs,
            "muTd": np.ascontiguousarray(muT_bf[:, k * CD:(k + 1) * CD]),
            "asel": asel,
            "bmask": bmask.astype(bf16),
        })
    return in_maps


def combine_outputs(results):
    outs = np.stack([np.asarray(r["out"]).astype(np.float64)
                     for r in results])                        # [8,128,4]
    comp_total = outs[:, :, 0].sum() - outs[:, :, 3].sum()
    dis_total = outs[:, :, 2].sum()
    loss_comp = comp_total / B
    loss_dis = np.log(1.0 / (C - 1)) + dis_total / C
    return np.array(ALPHA * loss_dis + LAMDA * loss_comp, dtype=np.float32)


def run_on_hw(z, target, mu, trace=False):
    nc = _get_program()
    in_maps = make_in_maps(z, target, mu)
    res = run_bass_kernel_spmd(nc, in_maps, core_ids=list(range(N_CORES)),
                               trace=trace)
    return combine_outputs(res.results), res


def kernel(z, target, mu):
    out, _ = run_on_hw(z, target, mu, trace=False)
    return out
